# revision 38
# baseline (speedup 1.0000x reference)
"""Trainium2 Bass kernel for nn_GatedShortBlock (gated depthwise-conv block).

Math (per batch b):
  BCx = x @ w1.T ; Bg, Cg, Xg = split(BCx, 3)
  gated = Bg * Xg
  conv  = causal depthwise conv1d(gated, conv_w, K=4)  (left pad 3)
  out   = (Cg * conv) @ w2.T

Sharding: data-parallel over (batch, seq-half) -> 8 shards of 2048 tokens.
Each core computes its shard fully on-device in a channel-major (transposed)
layout; the 3-token causal halo of `gated` at each shard start is computed
on the host (tiny) and shipped as an input.

v2: all matmul operands in bf16 (same PE rate as f32r, half the DMA bytes
and SBUF footprint), single token block per core (weights stream through
exactly once), host pre-tiles the weights into contiguous [128, KG*128]
slabs for clean DMA descriptors, x is loaded chunk-major and the matmul
loops are chunk-outer so the PE starts as soon as the first 2MB lands.
R (the mm2 rhs) overwrites the gated buffer in place.
"""

import sys

sys.path.insert(0, "/opt/trn_rl_repo")

import numpy as np
from contextlib import ExitStack

import concourse.bass as bass
import concourse.tile as tile
from concourse import bacc, mybir
from concourse.bass_utils import run_bass_kernel_spmd

F32 = mybir.dt.float32
BF16 = mybir.dt.bfloat16
NP_BF16 = mybir.dt.np(BF16)
KS = 4  # conv kernel size
HP = 8  # pad cols at the head of the gated buffer (last KS-1 hold the halo)


def build_program(D, E, T, CH, KG):
    """One-core program; SPMD across cores with different data."""
    ND = D // 128  # contraction tiles (d)
    NC = D // 128  # channel tiles (c) == output tiles (f)
    NE = E // 128  # w1 output tiles: [Bg: 0..NC-1, Cg: NC..2NC-1, Xg: 2NC..3NC-1]
    NCH = T // CH  # token chunks (psum-bank sized)
    NG = ND // KG  # weight slabs per output tile
    assert ND % KG == 0 and NE == 3 * NC and CH <= 512

    nc = bacc.Bacc(None)
    xT = nc.dram_tensor("xT", [D, T], BF16, kind="ExternalInput")
    # weight slab (e, g) at rows [(e*NG+g)*128 : +128]; slab[p, ks*128+m]
    #   = w[e*128+m, (g*KG+ks)*128+p]  (pre-tiled on host, fully contiguous)
    w1t = nc.dram_tensor("w1t", [NE * NG * 128, KG * 128], BF16, kind="ExternalInput")
    w2t = nc.dram_tensor("w2t", [NC * NG * 128, KG * 128], BF16, kind="ExternalInput")
    # conv weights / gated halo, host-prepacked so the SBUF tile matches a
    # fully contiguous DRAM block: cw[p, c*KS+j], gh[p, c*(KS-1)+j]
    cw = nc.dram_tensor("cw", [128, NC * KS], F32, kind="ExternalInput")
    gh = nc.dram_tensor("gh", [128, NC * (KS - 1)], BF16, kind="ExternalInput")
    outT = nc.dram_tensor("outT", [D, T], F32, kind="ExternalOutput")

    with tile.TileContext(nc) as tc, ExitStack() as ctx:
        xp = ctx.enter_context(tc.tile_pool(name="xp", bufs=1))
        gp = ctx.enter_context(tc.tile_pool(name="gp", bufs=1))
        w1p = ctx.enter_context(tc.tile_pool(name="w1p", bufs=15))
        w2p = ctx.enter_context(tc.tile_pool(name="w2p", bufs=4))
        sp = ctx.enter_context(tc.tile_pool(name="sp", bufs=2))
        stgp = ctx.enter_context(tc.tile_pool(name="stgp", bufs=4))
        smallp = ctx.enter_context(tc.tile_pool(name="smallp", bufs=1))
        psp = ctx.enter_context(tc.tile_pool(name="psp", bufs=8, space="PSUM"))

        # x tiles. DMA issues are FIFO per HWDGE ring (~0.6us each), so x is
        # split across both rings (sync + scalar); see the issue-order block
        # below for why ordering matters.
        xt = [xp.tile([128, T], BF16, tag=f"x{k}", name=f"x{k}") for k in range(ND)]

        X_ENGS = ("sync", "scalar")  # HWDGE issue rings for x loads

        def x_eng(k):
            return getattr(nc, X_ENGS[k % len(X_ENGS)])

        def x_load(u, width=1):
            for k in range(ND):
                x_eng(k).dma_start(
                    xt[k][:, u * CH : (u + width) * CH],
                    xT[k * 128 : (k + 1) * 128, u * CH : (u + width) * CH],
                )

        def w_slab(pool, wdram, idx, tag, eng=None):
            t = pool.tile([128, KG * 128], BF16, tag=tag, name=f"{tag}_{idx}")
            (eng or nc.sync).dma_start(t[:], wdram[idx * 128 : (idx + 1) * 128, :])
            return t

        def w1_slabs(e, eng=None):
            return [w_slab(w1p, w1t, e * NG + g, "w1", eng) for g in range(NG)]

        # critical path: x chunks 0-1 (one wide DMA per k) + c=0's B/X slabs
        # interleaved on the rings, then chunks 2-3 -- everything issued up
        # front so the ~0.6us/issue ring serialization never starves the PE.
        slabs0 = {}
        W01 = min(2, NCH)
        for k in range(ND):
            x_eng(k).dma_start(
                xt[k][:, 0 : W01 * CH],
                xT[k * 128 : (k + 1) * 128, 0 : W01 * CH],
            )
            if k == 0:
                slabs0[0] = [w_slab(w1p, w1t, g, "w1", nc.sync) for g in range(NG)]
            elif k == 1:
                slabs0[2 * NC] = [
                    w_slab(w1p, w1t, 2 * NC * NG + g, "w1", nc.scalar)
                    for g in range(NG)
                ]
        if NCH > W01:
            x_load(W01, width=NCH - W01)

        cwt = smallp.tile([128, NC * KS], F32, tag="cw", name="cw")
        ghall = smallp.tile([128, NC * (KS - 1)], BF16, tag="ghall", name="ghall")
        nc.sync.dma_start(cwt[:], cw[:, :])
        nc.scalar.dma_start(ghall[:], gh[:, :])

        def mm_chunk(ps, slabs, u):
            """ps += w[e-tile].T @ x[chunk u] (full contraction)."""
            for g in range(NG):
                for ks in range(KG):
                    k = g * KG + ks
                    nc.tensor.matmul(
                        ps[:],
                        slabs[g][:, ks * 128 : (ks + 1) * 128],
                        xt[k][:, u * CH : (u + 1) * CH],
                        start=(k == 0),
                        stop=(k == ND - 1),
                    )

        def mm_accum(pss, e):
            slabs = [w_slab(w1p, w1t, e * NG + g, "w1") for g in range(NG)]
            for u in range(NCH):
                mm_chunk(pss[u], slabs, u)

        gwork = [None] * NC

        def phaseA_setup(c):
            gw = gp.tile([128, HP + T], BF16, tag=f"g{c}", name=f"g{c}")
            gwork[c] = gw
            slabsB = slabs0[c] if c == 0 else w1_slabs(c)
            slabsX = slabs0[2 * NC] if c == 0 else w1_slabs(2 * NC + c)
            return (c, gw, slabsB, slabsX)

        def phaseA_chunk(st, u):
            # psum tiles created right before use: declaration order matches
            # use order, which keeps the slot allocator's schedule in line
            c, gw, slabsB, slabsX = st
            psB = psp.tile([128, CH], F32, tag="ps", name=f"psB{c}_{u}")
            psX = psp.tile([128, CH], F32, tag="ps", name=f"psX{c}_{u}")
            mm_chunk(psB, slabsB, u)
            mm_chunk(psX, slabsX, u)
            # DVE reads at most one PSUM operand per instruction:
            # stage Bg into gwork, then multiply Xg in place.
            dst = gw[:, HP + u * CH : HP + (u + 1) * CH]
            nc.vector.tensor_copy(dst, psB[:])
            nc.vector.tensor_mul(dst, dst, psX[:])

        def phaseB(c):
            # Cg, conv -> R (in place over gated)
            gw = gwork[c]
            nc.vector.tensor_copy(
                gw[:, HP - (KS - 1) : HP],
                ghall[:, c * (KS - 1) : (c + 1) * (KS - 1)],
            )
            psC = [
                psp.tile([128, CH], F32, tag="ps", name=f"psC{c}_{u}")
                for u in range(NCH)
            ]
            mm_accum(psC, NC + c)
            s = sp.tile([128, T], F32, tag="s", name=f"s0_{c}")
            nc.vector.tensor_scalar_mul(
                s[:], gw[:, HP - 3 : HP - 3 + T], cwt[:, c * KS : c * KS + 1]
            )
            for j in range(1, KS):
                s2 = sp.tile([128, T], F32, tag="s", name=f"s{j}_{c}")
                nc.vector.scalar_tensor_tensor(
                    s2[:],
                    gw[:, HP - 3 + j : HP - 3 + j + T],
                    cwt[:, c * KS + j : c * KS + j + 1],
                    s[:],
                    mybir.AluOpType.mult,
                    mybir.AluOpType.add,
                )
                s = s2
            for u in range(NCH):
                nc.vector.tensor_mul(
                    gw[:, HP + u * CH : HP + (u + 1) * CH],
                    s[:, u * CH : (u + 1) * CH],
                    psC[u][:],
                )

        for c in range(NC):
            with nc.named_scope(f"c{c}"):
                st = phaseA_setup(c)
                for u in range(NCH):
                    phaseA_chunk(st, u)
                phaseB(c)

        # ---- mm2: out = R.T @ w2.T (channel-major) ----
        def mm2_chunk(ps, slabs, col0, ncols):
            for g in range(NG):
                for cs in range(KG):
                    c = g * KG + cs
                    nc.tensor.matmul(
                        ps[:],
                        slabs[g][:, cs * 128 : (cs + 1) * 128],
                        gwork[c][:, HP + col0 : HP + col0 + ncols],
                        start=(c == 0),
                        stop=(c == NC - 1),
                    )

        def out_store(f, ps, col0, ncols, i):
            st = stgp.tile([128, ncols], F32, tag="stg", name=f"st{f}_{col0}")
            nc.vector.tensor_copy(st[:], ps[:, 0:ncols])
            eng = nc.sync if i % 2 == 0 else nc.scalar
            eng.dma_start(
                outT[f * 128 : (f + 1) * 128, col0 : col0 + ncols], st[:]
            )

        for f in range(NC):
            with nc.named_scope(f"f{f}"):
                slabs = [w_slab(w2p, w2t, f * NG + g, "w2") for g in range(NG)]
                last_f = f == NC - 1
                nfull = NCH - 1 if last_f else NCH
                ps2 = [
                    psp.tile([128, CH], F32, tag="ps", name=f"ps2{f}_{u}")
                    for u in range(nfull)
                ]
                for u in range(nfull):
                    mm2_chunk(ps2[u], slabs, u * CH, CH)
                if last_f:
                    # split the final chunk into 128-col pieces so the tail
                    # copy+DMA pipeline drains while the PE still accumulates
                    SUB = CH // 128
                    pss = [
                        psp.tile([128, 128], F32, tag="ps", name=f"ps2l_{s}")
                        for s in range(SUB)
                    ]
                    for s in range(SUB):
                        mm2_chunk(pss[s], slabs, (NCH - 1) * CH + s * 128, 128)
                for u in range(nfull):
                    out_store(f, ps2[u], u * CH, CH, u)
                if last_f:
                    for s in range(SUB):
                        out_store(f, pss[s], (NCH - 1) * CH + s * 128, 128, s)

    nc.finalize()
    return nc


def tile_weights(w, KG):
    """[M, K] -> slab layout [ (M/128)*(K/(KG*128))*128, KG*128 ],
    slab (e, g)[p, ks*128+m] = w[e*128+m, (g*KG+ks)*128+p]."""
    M, K = w.shape
    NE, NG = M // 128, K // (KG * 128)
    r = w.reshape(NE, 128, NG, KG, 128)  # [e, m, g, ks, p]
    r = r.transpose(0, 2, 4, 3, 1)  # [e, g, p, ks, m]
    return np.ascontiguousarray(r.reshape(NE * NG * 128, KG * 128).astype(NP_BF16))


def pack_per_channel(a, NC):
    """[D, J] -> [128, NC*J] with out[p, c*J+j] = a[c*128+p, j] (contiguous)."""
    D, J = a.shape
    return np.ascontiguousarray(
        a.reshape(NC, 128, J).transpose(1, 0, 2).reshape(128, NC * J)
    )


def shard_inputs(x, w1, w2, conv_w, D, T, KG):
    """Full inputs -> per-core in_maps (channel-major device layouts)."""
    B, S, _ = x.shape
    n_shards = (B * S) // T
    NC = D // 128
    w1t = tile_weights(w1, KG)
    w2t = tile_weights(w2, KG)
    cw = pack_per_channel(conv_w[:, 0, :].astype(np.float32), NC)
    shards_per_batch = S // T
    in_maps = []
    for s in range(n_shards):
        b, h = divmod(s, shards_per_batch)
        xs = x[b, h * T : (h + 1) * T, :]
        xTs = np.ascontiguousarray(xs.T.astype(NP_BF16))
        if h == 0:
            ghs = np.zeros((D, KS - 1), np.float32)
        else:
            xh = x[b, h * T - (KS - 1) : h * T, :]
            Bg = xh @ w1[0:D].T
            Xg = xh @ w1[2 * D : 3 * D].T
            ghs = (Bg * Xg).T
        ghp = pack_per_channel(ghs.astype(NP_BF16), NC)
        in_maps.append({"xT": xTs, "w1t": w1t, "w2t": w2t, "cw": cw, "gh": ghp})
    return in_maps


_PROGRAM_CACHE = {}


def run(x, w1, w2, conv_w, D=2048, T=2048, CH=512, KG=8, trace=False):
    B, S, _ = x.shape
    E = 3 * D
    key = (D, E, T, CH, KG)
    if key not in _PROGRAM_CACHE:
        _PROGRAM_CACHE[key] = build_program(D, E, T, CH, KG)
    nc = _PROGRAM_CACHE[key]
    in_maps = shard_inputs(x, w1, w2, conv_w, D, T, KG)
    n_shards = len(in_maps)
    res = run_bass_kernel_spmd(nc, in_maps, core_ids=list(range(n_shards)), trace=trace)
    shards_per_batch = S // T
    out = np.empty((B, S, D), np.float32)
    for s in range(n_shards):
        b, h = divmod(s, shards_per_batch)
        out[b, h * T : (h + 1) * T, :] = res.results[s]["outT"].T
    return out, res


def kernel(x, w1, w2, conv_w):
    x = np.asarray(x, np.float32)
    w1 = np.asarray(w1, np.float32)
    w2 = np.asarray(w2, np.float32)
    conv_w = np.asarray(conv_w, np.float32)
    out, _ = run(x, w1, w2, conv_w, D=2048, T=2048, CH=512, KG=8)
    return out


# revision 42
# speedup vs baseline: 1.0040x; 1.0040x over previous
"""Trainium2 Bass kernel for nn_GatedShortBlock (gated depthwise-conv block).

Math (per batch b):
  BCx = x @ w1.T ; Bg, Cg, Xg = split(BCx, 3)
  gated = Bg * Xg
  conv  = causal depthwise conv1d(gated, conv_w, K=4)  (left pad 3)
  out   = (Cg * conv) @ w2.T

Sharding: data-parallel over (batch, seq-half) -> 8 shards of 2048 tokens.
Each core computes its shard fully on-device in a channel-major (transposed)
layout; the 3-token causal halo of `gated` at each shard start is computed
on the host (tiny) and shipped as an input.

v2: all matmul operands in bf16 (same PE rate as f32r, half the DMA bytes
and SBUF footprint), single token block per core (weights stream through
exactly once), host pre-tiles the weights into contiguous [128, KG*128]
slabs for clean DMA descriptors, x is loaded chunk-major and the matmul
loops are chunk-outer so the PE starts as soon as the first 2MB lands.
R (the mm2 rhs) overwrites the gated buffer in place.
"""

import sys

sys.path.insert(0, "/opt/trn_rl_repo")

import numpy as np
from contextlib import ExitStack

import concourse.bass as bass
import concourse.tile as tile
from concourse import bacc, mybir
from concourse.bass_utils import run_bass_kernel_spmd

F32 = mybir.dt.float32
BF16 = mybir.dt.bfloat16
NP_BF16 = mybir.dt.np(BF16)
KS = 4  # conv kernel size
HP = 8  # pad cols at the head of the gated buffer (last KS-1 hold the halo)


def build_program(D, E, T, CH, KG):
    """One-core program; SPMD across cores with different data."""
    ND = D // 128  # contraction tiles (d)
    NC = D // 128  # channel tiles (c) == output tiles (f)
    NE = E // 128  # w1 output tiles: [Bg: 0..NC-1, Cg: NC..2NC-1, Xg: 2NC..3NC-1]
    NCH = T // CH  # token chunks (psum-bank sized)
    NG = ND // KG  # weight slabs per output tile
    assert ND % KG == 0 and NE == 3 * NC and CH <= 512

    nc = bacc.Bacc(None)
    xT = nc.dram_tensor("xT", [D, T], BF16, kind="ExternalInput")
    # weight slab (e, g) at rows [(e*NG+g)*128 : +128]; slab[p, ks*128+m]
    #   = w[e*128+m, (g*KG+ks)*128+p]  (pre-tiled on host, fully contiguous)
    w1t = nc.dram_tensor("w1t", [NE * NG * 128, KG * 128], BF16, kind="ExternalInput")
    w2t = nc.dram_tensor("w2t", [NC * NG * 128, KG * 128], BF16, kind="ExternalInput")
    # conv weights / gated halo, host-prepacked so the SBUF tile matches a
    # fully contiguous DRAM block: cw[p, c*KS+j], gh[p, c*(KS-1)+j]
    cw = nc.dram_tensor("cw", [128, NC * KS], F32, kind="ExternalInput")
    gh = nc.dram_tensor("gh", [128, NC * (KS - 1)], BF16, kind="ExternalInput")
    outT = nc.dram_tensor("outT", [D, T], F32, kind="ExternalOutput")

    with tile.TileContext(nc) as tc, ExitStack() as ctx:
        xp = ctx.enter_context(tc.tile_pool(name="xp", bufs=1))
        gp = ctx.enter_context(tc.tile_pool(name="gp", bufs=1))
        w1p = ctx.enter_context(tc.tile_pool(name="w1p", bufs=12))
        w2p = ctx.enter_context(tc.tile_pool(name="w2p", bufs=4))
        sp = ctx.enter_context(tc.tile_pool(name="sp", bufs=2))
        stgp = ctx.enter_context(tc.tile_pool(name="stgp", bufs=4))
        smallp = ctx.enter_context(tc.tile_pool(name="smallp", bufs=1))
        psp = ctx.enter_context(tc.tile_pool(name="psp", bufs=8, space="PSUM"))

        # x tiles. DMA issues are FIFO per HWDGE ring (~0.6us each), so x is
        # split across both rings (sync + scalar); see the issue-order block
        # below for why ordering matters.
        xt = [xp.tile([128, T], BF16, tag=f"x{k}", name=f"x{k}") for k in range(ND)]

        X_ENGS = ("sync", "scalar")  # HWDGE issue rings for x loads

        def x_eng(k):
            return getattr(nc, X_ENGS[k % len(X_ENGS)])

        def x_load(u, width=1):
            for k in range(ND):
                x_eng(k).dma_start(
                    xt[k][:, u * CH : (u + width) * CH],
                    xT[k * 128 : (k + 1) * 128, u * CH : (u + width) * CH],
                )

        def w_slab(pool, wdram, idx, tag, eng=None):
            t = pool.tile([128, KG * 128], BF16, tag=tag, name=f"{tag}_{idx}")
            (eng or nc.sync).dma_start(t[:], wdram[idx * 128 : (idx + 1) * 128, :])
            return t

        def w1_slabs(e, eng=None):
            return [w_slab(w1p, w1t, e * NG + g, "w1", eng) for g in range(NG)]

        # critical path: x[k=0,1] chunk0, then B/X slabs for BOTH c=0 and c=1
        # (the paired phase-A below consumes c=1's slabs at ~27us, and slab
        # DMAs queued behind the full x stream issue ~35us late -- the 8-sem
        # in-flight rotation throttles ring issue to ~1us/DMA), then the rest
        # of x chunk0 and the later chunks. All x loads are narrow 128KB so
        # transfers drain the in-flight window fast.
        slabs0 = {}
        NPAIR = min(2, NC)
        for k in range(2):
            x_eng(k).dma_start(
                xt[k][:, 0:CH], xT[k * 128 : (k + 1) * 128, 0:CH]
            )
        for c in range(NPAIR):
            slabs0[c] = [
                w_slab(w1p, w1t, c * NG + g, "w1", nc.sync) for g in range(NG)
            ]
            slabs0[2 * NC + c] = [
                w_slab(w1p, w1t, (2 * NC + c) * NG + g, "w1", nc.scalar)
                for g in range(NG)
            ]
        for k in range(2, ND):
            x_eng(k).dma_start(
                xt[k][:, 0:CH], xT[k * 128 : (k + 1) * 128, 0:CH]
            )
        if NCH > 1:
            x_load(1)

        cwt = smallp.tile([128, NC * KS], F32, tag="cw", name="cw")
        ghall = smallp.tile([128, NC * (KS - 1)], BF16, tag="ghall", name="ghall")
        nc.sync.dma_start(cwt[:], cw[:, :])
        nc.scalar.dma_start(ghall[:], gh[:, :])
        for u in range(2, NCH):
            x_load(u)

        def mm_chunk(ps, slabs, u):
            """ps += w[e-tile].T @ x[chunk u] (full contraction)."""
            for g in range(NG):
                for ks in range(KG):
                    k = g * KG + ks
                    nc.tensor.matmul(
                        ps[:],
                        slabs[g][:, ks * 128 : (ks + 1) * 128],
                        xt[k][:, u * CH : (u + 1) * CH],
                        start=(k == 0),
                        stop=(k == ND - 1),
                    )

        def mm_accum(pss, e):
            slabs = [w_slab(w1p, w1t, e * NG + g, "w1") for g in range(NG)]
            for u in range(NCH):
                mm_chunk(pss[u], slabs, u)

        gwork = [None] * NC

        def phaseA_setup(c):
            gw = gp.tile([128, HP + T], BF16, tag=f"g{c}", name=f"g{c}")
            gwork[c] = gw
            slabsB = slabs0.get(c) or w1_slabs(c)
            slabsX = slabs0.get(2 * NC + c) or w1_slabs(2 * NC + c)
            return (c, gw, slabsB, slabsX)

        def phaseA_chunk(st, u):
            # psum tiles created right before use: declaration order matches
            # use order, which keeps the slot allocator's schedule in line
            c, gw, slabsB, slabsX = st
            psB = psp.tile([128, CH], F32, tag="ps", name=f"psB{c}_{u}")
            psX = psp.tile([128, CH], F32, tag="ps", name=f"psX{c}_{u}")
            mm_chunk(psB, slabsB, u)
            mm_chunk(psX, slabsX, u)
            # DVE reads at most one PSUM operand per instruction:
            # stage Bg into gwork, then multiply Xg in place.
            dst = gw[:, HP + u * CH : HP + (u + 1) * CH]
            nc.vector.tensor_copy(dst, psB[:])
            nc.vector.tensor_mul(dst, dst, psX[:])

        def phaseB(c):
            # Cg, conv -> R (in place over gated)
            gw = gwork[c]
            nc.vector.tensor_copy(
                gw[:, HP - (KS - 1) : HP],
                ghall[:, c * (KS - 1) : (c + 1) * (KS - 1)],
            )
            psC = [
                psp.tile([128, CH], F32, tag="ps", name=f"psC{c}_{u}")
                for u in range(NCH)
            ]
            mm_accum(psC, NC + c)
            s = sp.tile([128, T], F32, tag="s", name=f"s0_{c}")
            nc.vector.tensor_scalar_mul(
                s[:], gw[:, HP - 3 : HP - 3 + T], cwt[:, c * KS : c * KS + 1]
            )
            for j in range(1, KS):
                s2 = sp.tile([128, T], F32, tag="s", name=f"s{j}_{c}")
                nc.vector.scalar_tensor_tensor(
                    s2[:],
                    gw[:, HP - 3 + j : HP - 3 + j + T],
                    cwt[:, c * KS + j : c * KS + j + 1],
                    s[:],
                    mybir.AluOpType.mult,
                    mybir.AluOpType.add,
                )
                s = s2
            for u in range(NCH):
                nc.vector.tensor_mul(
                    gw[:, HP + u * CH : HP + (u + 1) * CH],
                    s[:, u * CH : (u + 1) * CH],
                    psC[u][:],
                )

        # c=0,1: phase A interleaved at chunk level -- each x chunk feeds 4
        # accumulation passes (~13.6us of PE work per ~2.1MB chunk, 154GB/s
        # demand vs 358GB/s HBM), so the PE stream is dense from the first
        # matmul: no arrival crawl, no sparse window for HAM to re-throttle.
        with nc.named_scope("c01"):
            pair = [phaseA_setup(c) for c in range(NPAIR)]
            for u in range(NCH):
                for st in pair:
                    phaseA_chunk(st, u)
            for c in range(NPAIR):
                phaseB(c)
        for c in range(NPAIR, NC):
            with nc.named_scope(f"c{c}"):
                st = phaseA_setup(c)
                for u in range(NCH):
                    phaseA_chunk(st, u)
                phaseB(c)

        # ---- mm2: out = R.T @ w2.T (channel-major) ----
        def mm2_chunk(ps, slabs, col0, ncols):
            for g in range(NG):
                for cs in range(KG):
                    c = g * KG + cs
                    nc.tensor.matmul(
                        ps[:],
                        slabs[g][:, cs * 128 : (cs + 1) * 128],
                        gwork[c][:, HP + col0 : HP + col0 + ncols],
                        start=(c == 0),
                        stop=(c == NC - 1),
                    )

        def out_store(f, ps, col0, ncols, i):
            st = stgp.tile([128, ncols], F32, tag="stg", name=f"st{f}_{col0}")
            nc.vector.tensor_copy(st[:], ps[:, 0:ncols])
            eng = nc.sync if i % 2 == 0 else nc.scalar
            eng.dma_start(
                outT[f * 128 : (f + 1) * 128, col0 : col0 + ncols], st[:]
            )

        for f in range(NC):
            with nc.named_scope(f"f{f}"):
                slabs = [w_slab(w2p, w2t, f * NG + g, "w2") for g in range(NG)]
                last_f = f == NC - 1
                nfull = NCH - 1 if last_f else NCH
                ps2 = [
                    psp.tile([128, CH], F32, tag="ps", name=f"ps2{f}_{u}")
                    for u in range(nfull)
                ]
                for u in range(nfull):
                    mm2_chunk(ps2[u], slabs, u * CH, CH)
                if last_f:
                    # split the final chunk into 128-col pieces so the tail
                    # copy+DMA pipeline drains while the PE still accumulates
                    SUB = CH // 128
                    pss = [
                        psp.tile([128, 128], F32, tag="ps", name=f"ps2l_{s}")
                        for s in range(SUB)
                    ]
                    for s in range(SUB):
                        mm2_chunk(pss[s], slabs, (NCH - 1) * CH + s * 128, 128)
                for u in range(nfull):
                    out_store(f, ps2[u], u * CH, CH, u)
                if last_f:
                    for s in range(SUB):
                        out_store(f, pss[s], (NCH - 1) * CH + s * 128, 128, s)

    nc.finalize()
    return nc


def tile_weights(w, KG):
    """[M, K] -> slab layout [ (M/128)*(K/(KG*128))*128, KG*128 ],
    slab (e, g)[p, ks*128+m] = w[e*128+m, (g*KG+ks)*128+p]."""
    M, K = w.shape
    NE, NG = M // 128, K // (KG * 128)
    r = w.reshape(NE, 128, NG, KG, 128)  # [e, m, g, ks, p]
    r = r.transpose(0, 2, 4, 3, 1)  # [e, g, p, ks, m]
    return np.ascontiguousarray(r.reshape(NE * NG * 128, KG * 128).astype(NP_BF16))


def pack_per_channel(a, NC):
    """[D, J] -> [128, NC*J] with out[p, c*J+j] = a[c*128+p, j] (contiguous)."""
    D, J = a.shape
    return np.ascontiguousarray(
        a.reshape(NC, 128, J).transpose(1, 0, 2).reshape(128, NC * J)
    )


def shard_inputs(x, w1, w2, conv_w, D, T, KG):
    """Full inputs -> per-core in_maps (channel-major device layouts)."""
    B, S, _ = x.shape
    n_shards = (B * S) // T
    NC = D // 128
    w1t = tile_weights(w1, KG)
    w2t = tile_weights(w2, KG)
    cw = pack_per_channel(conv_w[:, 0, :].astype(np.float32), NC)
    shards_per_batch = S // T
    in_maps = []
    for s in range(n_shards):
        b, h = divmod(s, shards_per_batch)
        xs = x[b, h * T : (h + 1) * T, :]
        xTs = np.ascontiguousarray(xs.T.astype(NP_BF16))
        if h == 0:
            ghs = np.zeros((D, KS - 1), np.float32)
        else:
            xh = x[b, h * T - (KS - 1) : h * T, :]
            Bg = xh @ w1[0:D].T
            Xg = xh @ w1[2 * D : 3 * D].T
            ghs = (Bg * Xg).T
        ghp = pack_per_channel(ghs.astype(NP_BF16), NC)
        in_maps.append({"xT": xTs, "w1t": w1t, "w2t": w2t, "cw": cw, "gh": ghp})
    return in_maps


_PROGRAM_CACHE = {}


def run(x, w1, w2, conv_w, D=2048, T=2048, CH=512, KG=8, trace=False):
    B, S, _ = x.shape
    E = 3 * D
    key = (D, E, T, CH, KG)
    if key not in _PROGRAM_CACHE:
        _PROGRAM_CACHE[key] = build_program(D, E, T, CH, KG)
    nc = _PROGRAM_CACHE[key]
    in_maps = shard_inputs(x, w1, w2, conv_w, D, T, KG)
    n_shards = len(in_maps)
    res = run_bass_kernel_spmd(nc, in_maps, core_ids=list(range(n_shards)), trace=trace)
    shards_per_batch = S // T
    out = np.empty((B, S, D), np.float32)
    for s in range(n_shards):
        b, h = divmod(s, shards_per_batch)
        out[b, h * T : (h + 1) * T, :] = res.results[s]["outT"].T
    return out, res


def kernel(x, w1, w2, conv_w):
    x = np.asarray(x, np.float32)
    w1 = np.asarray(w1, np.float32)
    w2 = np.asarray(w2, np.float32)
    conv_w = np.asarray(conv_w, np.float32)
    out, _ = run(x, w1, w2, conv_w, D=2048, T=2048, CH=512, KG=8)
    return out


# revision 43
# speedup vs baseline: 1.0044x; 1.0004x over previous
"""Trainium2 Bass kernel for nn_GatedShortBlock (gated depthwise-conv block).

Math (per batch b):
  BCx = x @ w1.T ; Bg, Cg, Xg = split(BCx, 3)
  gated = Bg * Xg
  conv  = causal depthwise conv1d(gated, conv_w, K=4)  (left pad 3)
  out   = (Cg * conv) @ w2.T

Sharding: data-parallel over (batch, seq-half) -> 8 shards of 2048 tokens.
Each core computes its shard fully on-device in a channel-major (transposed)
layout; the 3-token causal halo of `gated` at each shard start is computed
on the host (tiny) and shipped as an input.

v2: all matmul operands in bf16 (same PE rate as f32r, half the DMA bytes
and SBUF footprint), single token block per core (weights stream through
exactly once), host pre-tiles the weights into contiguous [128, KG*128]
slabs for clean DMA descriptors, x is loaded chunk-major and the matmul
loops are chunk-outer so the PE starts as soon as the first 2MB lands.
R (the mm2 rhs) overwrites the gated buffer in place.
"""

import sys

sys.path.insert(0, "/opt/trn_rl_repo")

import numpy as np
from contextlib import ExitStack

import concourse.bass as bass
import concourse.tile as tile
from concourse import bacc, mybir
from concourse.bass_utils import run_bass_kernel_spmd

F32 = mybir.dt.float32
BF16 = mybir.dt.bfloat16
NP_BF16 = mybir.dt.np(BF16)
KS = 4  # conv kernel size
HP = 8  # pad cols at the head of the gated buffer (last KS-1 hold the halo)


def build_program(D, E, T, CH, KG):
    """One-core program; SPMD across cores with different data."""
    ND = D // 128  # contraction tiles (d)
    NC = D // 128  # channel tiles (c) == output tiles (f)
    NE = E // 128  # w1 output tiles: [Bg: 0..NC-1, Cg: NC..2NC-1, Xg: 2NC..3NC-1]
    NCH = T // CH  # token chunks (psum-bank sized)
    NG = ND // KG  # weight slabs per output tile
    assert ND % KG == 0 and NE == 3 * NC and CH <= 512

    nc = bacc.Bacc(None)
    xT = nc.dram_tensor("xT", [D, T], BF16, kind="ExternalInput")
    # weight slab (e, g) at rows [(e*NG+g)*128 : +128]; slab[p, ks*128+m]
    #   = w[e*128+m, (g*KG+ks)*128+p]  (pre-tiled on host, fully contiguous)
    w1t = nc.dram_tensor("w1t", [NE * NG * 128, KG * 128], BF16, kind="ExternalInput")
    w2t = nc.dram_tensor("w2t", [NC * NG * 128, KG * 128], BF16, kind="ExternalInput")
    # conv weights / gated halo, host-prepacked so the SBUF tile matches a
    # fully contiguous DRAM block: cw[p, c*KS+j], gh[p, c*(KS-1)+j]
    cw = nc.dram_tensor("cw", [128, NC * KS], F32, kind="ExternalInput")
    gh = nc.dram_tensor("gh", [128, NC * (KS - 1)], BF16, kind="ExternalInput")
    outT = nc.dram_tensor("outT", [D, T], F32, kind="ExternalOutput")

    with tile.TileContext(nc) as tc, ExitStack() as ctx:
        xp = ctx.enter_context(tc.tile_pool(name="xp", bufs=1))
        gp = ctx.enter_context(tc.tile_pool(name="gp", bufs=1))
        w1p = ctx.enter_context(tc.tile_pool(name="w1p", bufs=12))
        w2p = ctx.enter_context(tc.tile_pool(name="w2p", bufs=4))
        sp = ctx.enter_context(tc.tile_pool(name="sp", bufs=2))
        stgp = ctx.enter_context(tc.tile_pool(name="stgp", bufs=4))
        smallp = ctx.enter_context(tc.tile_pool(name="smallp", bufs=1))
        psp = ctx.enter_context(tc.tile_pool(name="psp", bufs=8, space="PSUM"))

        # x tiles. DMA issues are FIFO per HWDGE ring (~0.6us each), so x is
        # split across both rings (sync + scalar); see the issue-order block
        # below for why ordering matters.
        xt = [xp.tile([128, T], BF16, tag=f"x{k}", name=f"x{k}") for k in range(ND)]

        X_ENGS = ("sync", "scalar")  # HWDGE issue rings for x loads

        def x_eng(k):
            return getattr(nc, X_ENGS[k % len(X_ENGS)])

        def x_load(u, width=1):
            for k in range(ND):
                x_eng(k).dma_start(
                    xt[k][:, u * CH : (u + width) * CH],
                    xT[k * 128 : (k + 1) * 128, u * CH : (u + width) * CH],
                )

        def w_slab(pool, wdram, idx, tag, eng=None):
            t = pool.tile([128, KG * 128], BF16, tag=tag, name=f"{tag}_{idx}")
            (eng or nc.sync).dma_start(t[:], wdram[idx * 128 : (idx + 1) * 128, :])
            return t

        def w1_slabs(e, eng=None):
            return [w_slab(w1p, w1t, e * NG + g, "w1", eng) for g in range(NG)]

        # critical path: x[k=0,1] chunk0, then B/X slabs for BOTH c=0 and c=1
        # (the paired phase-A below consumes c=1's slabs at ~27us, and slab
        # DMAs queued behind the full x stream issue ~35us late -- the 8-sem
        # in-flight rotation throttles ring issue to ~1us/DMA), then the rest
        # of x chunk0 and the later chunks. All x loads are narrow 128KB so
        # transfers drain the in-flight window fast.
        slabs0 = {}
        NPAIR = min(2, NC)
        for k in range(2):
            x_eng(k).dma_start(
                xt[k][:, 0:CH], xT[k * 128 : (k + 1) * 128, 0:CH]
            )
        # c0's B/X slabs next (first two passes); c1's ride AFTER the x
        # chunk0 tiles -- they aren't read until ~20us and each ring slot
        # they'd occupy here delays the delivery-bound x k-tail by ~0.6us
        slabs0[0] = [w_slab(w1p, w1t, g, "w1", nc.sync) for g in range(NG)]
        slabs0[2 * NC] = [
            w_slab(w1p, w1t, 2 * NC * NG + g, "w1", nc.scalar) for g in range(NG)
        ]
        for k in range(2, ND):
            x_eng(k).dma_start(
                xt[k][:, 0:CH], xT[k * 128 : (k + 1) * 128, 0:CH]
            )
        if NPAIR > 1:
            slabs0[1] = [
                w_slab(w1p, w1t, NG + g, "w1", nc.sync) for g in range(NG)
            ]
            slabs0[2 * NC + 1] = [
                w_slab(w1p, w1t, (2 * NC + 1) * NG + g, "w1", nc.scalar)
                for g in range(NG)
            ]
        if NCH > 1:
            x_load(1)

        cwt = smallp.tile([128, NC * KS], F32, tag="cw", name="cw")
        ghall = smallp.tile([128, NC * (KS - 1)], BF16, tag="ghall", name="ghall")
        nc.sync.dma_start(cwt[:], cw[:, :])
        nc.scalar.dma_start(ghall[:], gh[:, :])
        for u in range(2, NCH):
            x_load(u)

        def mm_chunk(ps, slabs, u):
            """ps += w[e-tile].T @ x[chunk u] (full contraction)."""
            for g in range(NG):
                for ks in range(KG):
                    k = g * KG + ks
                    nc.tensor.matmul(
                        ps[:],
                        slabs[g][:, ks * 128 : (ks + 1) * 128],
                        xt[k][:, u * CH : (u + 1) * CH],
                        start=(k == 0),
                        stop=(k == ND - 1),
                    )

        def mm_accum(pss, e):
            slabs = [w_slab(w1p, w1t, e * NG + g, "w1") for g in range(NG)]
            for u in range(NCH):
                mm_chunk(pss[u], slabs, u)

        gwork = [None] * NC

        def phaseA_setup(c):
            gw = gp.tile([128, HP + T], BF16, tag=f"g{c}", name=f"g{c}")
            gwork[c] = gw
            slabsB = slabs0.get(c) or w1_slabs(c)
            slabsX = slabs0.get(2 * NC + c) or w1_slabs(2 * NC + c)
            return (c, gw, slabsB, slabsX)

        def phaseA_chunk(st, u):
            # psum tiles created right before use: declaration order matches
            # use order, which keeps the slot allocator's schedule in line
            c, gw, slabsB, slabsX = st
            psB = psp.tile([128, CH], F32, tag="ps", name=f"psB{c}_{u}")
            psX = psp.tile([128, CH], F32, tag="ps", name=f"psX{c}_{u}")
            mm_chunk(psB, slabsB, u)
            mm_chunk(psX, slabsX, u)
            # DVE reads at most one PSUM operand per instruction:
            # stage Bg into gwork, then multiply Xg in place.
            dst = gw[:, HP + u * CH : HP + (u + 1) * CH]
            nc.vector.tensor_copy(dst, psB[:])
            nc.vector.tensor_mul(dst, dst, psX[:])

        def phaseB(c):
            # Cg, conv -> R (in place over gated)
            gw = gwork[c]
            nc.vector.tensor_copy(
                gw[:, HP - (KS - 1) : HP],
                ghall[:, c * (KS - 1) : (c + 1) * (KS - 1)],
            )
            psC = [
                psp.tile([128, CH], F32, tag="ps", name=f"psC{c}_{u}")
                for u in range(NCH)
            ]
            mm_accum(psC, NC + c)
            s = sp.tile([128, T], F32, tag="s", name=f"s0_{c}")
            nc.vector.tensor_scalar_mul(
                s[:], gw[:, HP - 3 : HP - 3 + T], cwt[:, c * KS : c * KS + 1]
            )
            for j in range(1, KS):
                s2 = sp.tile([128, T], F32, tag="s", name=f"s{j}_{c}")
                nc.vector.scalar_tensor_tensor(
                    s2[:],
                    gw[:, HP - 3 + j : HP - 3 + j + T],
                    cwt[:, c * KS + j : c * KS + j + 1],
                    s[:],
                    mybir.AluOpType.mult,
                    mybir.AluOpType.add,
                )
                s = s2
            for u in range(NCH):
                nc.vector.tensor_mul(
                    gw[:, HP + u * CH : HP + (u + 1) * CH],
                    s[:, u * CH : (u + 1) * CH],
                    psC[u][:],
                )

        # c=0,1: phase A interleaved at chunk level -- each x chunk feeds 4
        # accumulation passes (~13.6us of PE work per ~2.1MB chunk, 154GB/s
        # demand vs 358GB/s HBM), so the PE stream is dense from the first
        # matmul: no arrival crawl, no sparse window for HAM to re-throttle.
        with nc.named_scope("c01"):
            pair = [phaseA_setup(c) for c in range(NPAIR)]
            for u in range(NCH):
                for st in pair:
                    phaseA_chunk(st, u)
            for c in range(NPAIR):
                phaseB(c)
        for c in range(NPAIR, NC):
            with nc.named_scope(f"c{c}"):
                st = phaseA_setup(c)
                for u in range(NCH):
                    phaseA_chunk(st, u)
                phaseB(c)

        # ---- mm2: out = R.T @ w2.T (channel-major) ----
        def mm2_chunk(ps, slabs, col0, ncols):
            for g in range(NG):
                for cs in range(KG):
                    c = g * KG + cs
                    nc.tensor.matmul(
                        ps[:],
                        slabs[g][:, cs * 128 : (cs + 1) * 128],
                        gwork[c][:, HP + col0 : HP + col0 + ncols],
                        start=(c == 0),
                        stop=(c == NC - 1),
                    )

        def out_store(f, ps, col0, ncols, i):
            st = stgp.tile([128, ncols], F32, tag="stg", name=f"st{f}_{col0}")
            nc.vector.tensor_copy(st[:], ps[:, 0:ncols])
            eng = nc.sync if i % 2 == 0 else nc.scalar
            eng.dma_start(
                outT[f * 128 : (f + 1) * 128, col0 : col0 + ncols], st[:]
            )

        for f in range(NC):
            with nc.named_scope(f"f{f}"):
                slabs = [w_slab(w2p, w2t, f * NG + g, "w2") for g in range(NG)]
                last_f = f == NC - 1
                nfull = NCH - 1 if last_f else NCH
                ps2 = [
                    psp.tile([128, CH], F32, tag="ps", name=f"ps2{f}_{u}")
                    for u in range(nfull)
                ]
                for u in range(nfull):
                    mm2_chunk(ps2[u], slabs, u * CH, CH)
                if last_f:
                    # split the final chunk into 128-col pieces so the tail
                    # copy+DMA pipeline drains while the PE still accumulates
                    SUB = CH // 128
                    pss = [
                        psp.tile([128, 128], F32, tag="ps", name=f"ps2l_{s}")
                        for s in range(SUB)
                    ]
                    for s in range(SUB):
                        mm2_chunk(pss[s], slabs, (NCH - 1) * CH + s * 128, 128)
                for u in range(nfull):
                    out_store(f, ps2[u], u * CH, CH, u)
                if last_f:
                    for s in range(SUB):
                        out_store(f, pss[s], (NCH - 1) * CH + s * 128, 128, s)

    nc.finalize()
    return nc


def tile_weights(w, KG):
    """[M, K] -> slab layout [ (M/128)*(K/(KG*128))*128, KG*128 ],
    slab (e, g)[p, ks*128+m] = w[e*128+m, (g*KG+ks)*128+p]."""
    M, K = w.shape
    NE, NG = M // 128, K // (KG * 128)
    r = w.reshape(NE, 128, NG, KG, 128)  # [e, m, g, ks, p]
    r = r.transpose(0, 2, 4, 3, 1)  # [e, g, p, ks, m]
    return np.ascontiguousarray(r.reshape(NE * NG * 128, KG * 128).astype(NP_BF16))


def pack_per_channel(a, NC):
    """[D, J] -> [128, NC*J] with out[p, c*J+j] = a[c*128+p, j] (contiguous)."""
    D, J = a.shape
    return np.ascontiguousarray(
        a.reshape(NC, 128, J).transpose(1, 0, 2).reshape(128, NC * J)
    )


def shard_inputs(x, w1, w2, conv_w, D, T, KG):
    """Full inputs -> per-core in_maps (channel-major device layouts)."""
    B, S, _ = x.shape
    n_shards = (B * S) // T
    NC = D // 128
    w1t = tile_weights(w1, KG)
    w2t = tile_weights(w2, KG)
    cw = pack_per_channel(conv_w[:, 0, :].astype(np.float32), NC)
    shards_per_batch = S // T
    in_maps = []
    for s in range(n_shards):
        b, h = divmod(s, shards_per_batch)
        xs = x[b, h * T : (h + 1) * T, :]
        xTs = np.ascontiguousarray(xs.T.astype(NP_BF16))
        if h == 0:
            ghs = np.zeros((D, KS - 1), np.float32)
        else:
            xh = x[b, h * T - (KS - 1) : h * T, :]
            Bg = xh @ w1[0:D].T
            Xg = xh @ w1[2 * D : 3 * D].T
            ghs = (Bg * Xg).T
        ghp = pack_per_channel(ghs.astype(NP_BF16), NC)
        in_maps.append({"xT": xTs, "w1t": w1t, "w2t": w2t, "cw": cw, "gh": ghp})
    return in_maps


_PROGRAM_CACHE = {}


def run(x, w1, w2, conv_w, D=2048, T=2048, CH=512, KG=8, trace=False):
    B, S, _ = x.shape
    E = 3 * D
    key = (D, E, T, CH, KG)
    if key not in _PROGRAM_CACHE:
        _PROGRAM_CACHE[key] = build_program(D, E, T, CH, KG)
    nc = _PROGRAM_CACHE[key]
    in_maps = shard_inputs(x, w1, w2, conv_w, D, T, KG)
    n_shards = len(in_maps)
    res = run_bass_kernel_spmd(nc, in_maps, core_ids=list(range(n_shards)), trace=trace)
    shards_per_batch = S // T
    out = np.empty((B, S, D), np.float32)
    for s in range(n_shards):
        b, h = divmod(s, shards_per_batch)
        out[b, h * T : (h + 1) * T, :] = res.results[s]["outT"].T
    return out, res


def kernel(x, w1, w2, conv_w):
    x = np.asarray(x, np.float32)
    w1 = np.asarray(w1, np.float32)
    w2 = np.asarray(w2, np.float32)
    conv_w = np.asarray(conv_w, np.float32)
    out, _ = run(x, w1, w2, conv_w, D=2048, T=2048, CH=512, KG=8)
    return out
